# revision 9
# baseline (speedup 1.0000x reference)
"""Bass/Tile kernel for nn_CrossAttention (retrieval_knn):
out = softmax(-cdist(Q, K) / 8, axis=-1), Q/K: [4, 4096, 64] fp32.

Sharding: 16384 query rows across 8 cores (2048 rows/core = half a batch);
K replicated per batch (cores 2b, 2b+1 get K[b]).

Per-core pipeline (rows=2048 -> 16 row-tiles of 128):
  PE:  psum[n,m] = d2[n,m] via ONE K=66 extended f32r matmul:
       qt rows = [-2*q^T; ones; q2], kt rows = [k^T; k2; ones]
       (q2/k2 precomputed on host), so psum = q2 + k2 - 2 q.k = d2 >= 0.
  sqrt s = sqrt(d2) -> fp16 s tiles in SBUF, split across two engines:
    ACT tiles: activation Sqrt from PSUM (sqrt table, loaded once)
    DVE tiles (dve_tiles of them, interleaved): bit-hack rsqrt seed
      r0 = bitcast((~bits(d2)) >> 1)  [one tensor_scalar, xor+shift]
      then one fused custom DVE op (Newton step + final mul):
      s = (c0*r0) * (c1 - c2 * ((c0*r0)^2 * d2)) * d2   (~0.18% rel)
  ACT: e = exp(-s/8) fp16 in-place over s, accum_out -> row sums (exp table
       loaded once; phase-ordered after the last ACT sqrt)
  DVE: r = 1/sums; e *= r (fp16 4x mode)
  DMA: store [128, 4096] fp16 (1 MiB) per row-tile; host casts to fp32.
"""

import sys
import numpy as np

try:
    import concourse.bass as bass  # noqa: F401
except ImportError:  # container staging path
    sys.path.insert(0, "/opt/trn_rl_repo")
    import concourse.bass as bass  # noqa: F401

import concourse.mybir as mybir
import concourse.tile as tile
from concourse import bacc
from concourse.bass import ts
from concourse.bass_utils import run_bass_kernel_spmd
from concourse.dve_ops import (
    CUSTOM_DVE_SPECS,
    OPS,
    DveOp,
    _CUSTOM_DVE_ROW_BASE,
    _SUB_OPCODE_FOR_NAME,
)
from concourse.dve_spec import C0, C1, C2, Spec, Src0, Src1, lower, sq
from concourse.dve_uop import DveOpSpec
from concourse.tile import add_dep_helper

F32 = mybir.dt.float32
F32R = mybir.dt.float32r
F16 = mybir.dt.float16
BF16 = mybir.dt.bfloat16
I32 = mybir.dt.int32
AF = mybir.ActivationFunctionType
ALU = mybir.AluOpType

B, N, M, D = 4, 4096, 4096, 64
N_CORES = 8
ROWS = B * N // N_CORES  # 2048 query rows per core
KDIM = D + 2  # 66: q rows + ones + q2 (resp. k rows + k2 + ones)

# constants for the DVE sqrt path, fit over d2 in [30, 310] (valid for any
# positive normal fp32 input range; sawtooth is log-periodic)
SQRT_C0 = -1.8355344587704822e-20
SQRT_C1 = 1.5013519525527954
SQRT_C2 = 0.5000000596046448

DEFAULT_KW = dict(dvew=0)


def _register_sqrt_op():
    name = "SQRT_NR_ANT"
    for op in OPS:
        if op.name == name:
            return op
    _r0s = Src0 * C0
    body = (_r0s * (C1 - C2 * (sq(_r0s) * Src1))) * Src1
    spec = Spec(
        body=body,
        reference=lambda in0, in1, s0, s1, imm2: (
            ((in0 * s0) * (s1 - imm2 * ((in0 * s0) ** 2 * in1))) * in1
        ).astype(np.float32),
    )
    row = _CUSTOM_DVE_ROW_BASE + len(OPS)
    shas = {
        v: DveOpSpec(name=name, opcode=row, uops=lower(spec, ver=v), rd1_en=True).sha(v)
        for v in ("v3", "v4")
    }
    op = DveOp(name, spec, subdim=False, uops_sha=shas)
    OPS.append(op)
    _SUB_OPCODE_FOR_NAME[name] = row
    CUSTOM_DVE_SPECS[name] = spec
    return op


SQRT_NR_OP = _register_sqrt_op()


def round_f32r(x):
    """fp32 -> fp32r rounding (RNE at mantissa bit 12), matching the PE."""
    u = np.ascontiguousarray(x, np.float32).view(np.uint32)
    lo = u & np.uint32(0xFFF)
    hi = u & np.uint32(0xFFFFF000)
    up = (lo > 0x800) | ((lo == 0x800) & (((u >> np.uint32(12)) & np.uint32(1)) == 1))
    return (hi + np.where(up, np.uint32(0x1000), np.uint32(0))).view(np.float32)


def build_kernel(rows=ROWS, m=M, dvew=1024, reps=1, sbufs=17):
    """dvew: number of columns per row-tile whose sqrt runs on the DVE
    (seed + fused Newton) instead of ACT. 0 disables the DVE path."""
    assert rows % 128 == 0 and m % 512 == 0 and dvew % 512 == 0 and dvew <= m
    n_tiles = rows // 128
    # chunk widths + psum pool depths (PSUM budget: 8 banks x 2KB/partition)
    if dvew <= 1536:
        d_w, d_bufs = dvew, 2          # 2*dvew/512 banks
        wa_max = 2048 - dvew           # 2*wa/512 banks
        a_bufs = 2
    elif dvew == 2048:
        d_w, d_bufs = 1024, 2          # 4 banks
        wa_max, a_bufs = 1024, 2       # 4 banks
    else:
        d_w, d_bufs = 512, 4           # 4 banks (6 when no a-pool)
        wa_max, a_bufs = 512, 2        # 2 banks
        if dvew == m:
            d_bufs = 6
    d_chunks = []
    off = 0
    while off < dvew:
        w = min(d_w, dvew - off)
        d_chunks.append((off, w))
        off += w
    a_chunks = []
    while off < m:
        w = min(wa_max, m - off)
        a_chunks.append((off, w))
        off += w

    nc = bacc.Bacc("TRN2", target_bir_lowering=False, debug=False)
    qt = nc.dram_tensor("qt", [KDIM, rows], F32R, kind="ExternalInput")
    kt = nc.dram_tensor("kt", [KDIM, m], F32R, kind="ExternalInput")
    out = nc.dram_tensor("out", [rows, m], F16, kind="ExternalOutput")

    with tile.TileContext(nc) as tc:
        with (
            tc.tile_pool(name="const", bufs=1) as cpool,
            tc.tile_pool(name="spool", bufs=sbufs) as spool,
            tc.tile_pool(name="seed", bufs=max(2, d_bufs)) as dpool,
            tc.tile_pool(name="psum_a", bufs=a_bufs, space="PSUM") as ppa,
            tc.tile_pool(name="psum_d", bufs=d_bufs, space="PSUM") as ppd,
        ):
          for _rep in range(reps):
            qe = cpool.tile([KDIM, rows], F32R, name="qe")
            ke = cpool.tile([KDIM, m], F32R, name="ke")
            nc.sync.dma_start(out=qe[:, :], in_=qt[:, :])
            for c in range(4):
                nc.sync.dma_start(out=ke[:, ts(c, m // 4)], in_=kt[:, ts(c, m // 4)])
            sums = cpool.tile([128, n_tiles], F32, name="sums")
            recs = cpool.tile([128, n_tiles], F32, name="recs")

            s_tiles = [None] * n_tiles
            sqrt_acts = []
            for t in range(n_tiles):
                s_t = spool.tile([128, m], F16, tag="s", name="s_t")
                s_tiles[t] = s_t
                for off, w in d_chunks:
                    pd = ppd.tile([128, w], F32, tag="pd", name="pd")
                    for j in range(w // 512):
                        nc.tensor.matmul(
                            pd[:, ts(j, 512)],
                            qe[:, ts(t, 128)],
                            ke[:, ts(off // 512 + j, 512)],
                            start=True, stop=True,
                        )
                    sd = dpool.tile([128, w], F32, name="sd")
                    nc.vector.tensor_scalar(
                        out=sd.bitcast(I32)[:, :], in0=pd.bitcast(I32)[:, :],
                        scalar1=-1, scalar2=1,
                        op0=ALU.bitwise_xor, op1=ALU.arith_shift_right,
                    )
                    nc.vector._custom_dve(
                        SQRT_NR_OP, out=s_t[:, off : off + w],
                        in0=sd[:, :], in1=pd[:, :],
                        s0=SQRT_C0, s1=SQRT_C1, imm2=SQRT_C2,
                    )
                for off, w in a_chunks:
                    pa = ppa.tile([128, w], F32, tag="pa", name="pa")
                    for j in range(w // 512):
                        nc.tensor.matmul(
                            pa[:, ts(j, 512)],
                            qe[:, ts(t, 128)],
                            ke[:, ts(off // 512 + j, 512)],
                            start=True, stop=True,
                        )
                    act = nc.scalar.activation(
                        out=s_t[:, off : off + w], in_=pa[:, :], func=AF.Sqrt,
                    )
                    sqrt_acts.append(act)
            last_sqrt = sqrt_acts[-1] if sqrt_acts else None

            for t in range(n_tiles):
                e_v = s_tiles[t]  # exp fp16 -> fp16 in place
                e = nc.scalar.activation(
                    out=e_v[:, :], in_=s_tiles[t][:, :], func=AF.Exp,
                    scale=-0.125, accum_out=sums[:, t : t + 1],
                )
                if last_sqrt is not None:
                    add_dep_helper(e.ins, last_sqrt.ins, False, "act-table phase order")
                nc.vector.reciprocal(out=recs[:, t : t + 1], in_=sums[:, t : t + 1])
                nc.vector.tensor_scalar_mul(e_v[:, :], e_v[:, :], recs[:, t : t + 1])
                nc.sync.dma_start(out=out[ts(t, 128), :], in_=e_v[:, :])
    nc.compile()
    return nc


def make_in_maps(Q, K):
    Q = np.asarray(Q, dtype=np.float32)
    K = np.asarray(K, dtype=np.float32)
    in_maps = []
    for i in range(N_CORES):
        b, h = divmod(i, N_CORES // B)
        qs = Q[b, h * ROWS : (h + 1) * ROWS]  # [2048, 64]
        ks = K[b]                             # [4096, 64]
        qr = round_f32r(qs)
        kr = round_f32r(ks)
        q2 = round_f32r((qr.astype(np.float64) ** 2).sum(1).astype(np.float32))
        k2 = round_f32r((kr.astype(np.float64) ** 2).sum(1).astype(np.float32))
        ones_q = np.ones((1, qr.shape[0]), np.float32)
        ones_k = np.ones((1, kr.shape[0]), np.float32)
        qt_ext = np.concatenate([(-2.0 * qr).T, ones_q, q2[None, :]], axis=0)
        kt_ext = np.concatenate([kr.T, k2[None, :], ones_k], axis=0)
        in_maps.append({
            "qt": np.ascontiguousarray(qt_ext),
            "kt": np.ascontiguousarray(kt_ext),
        })
    return in_maps


_NC_CACHE = {}


def get_nc(**kw):
    key = tuple(sorted(kw.items()))
    if key not in _NC_CACHE:
        _NC_CACHE[key] = build_kernel(**kw)
    return _NC_CACHE[key]


def kernel(Q, K):
    nc = get_nc(**DEFAULT_KW)
    in_maps = make_in_maps(Q, K)
    res = run_bass_kernel_spmd(nc, in_maps, core_ids=list(range(N_CORES)))
    out = np.empty((B, N, M), dtype=np.float32)
    for i in range(N_CORES):
        b, h = divmod(i, N_CORES // B)
        out[b, h * ROWS : (h + 1) * ROWS] = np.asarray(
            res.results[i]["out"]
        ).astype(np.float32)
    return out
